# revision 22
# baseline (speedup 1.0000x reference)
"""MoE routed-expert kernel for Trainium2 (8 NeuronCores, SPMD).

Problem: N=16384 tokens, D=768, H=768, C=2, E=20 experts.
  y[n] = relu(x[n] @ W1[e] + b1[e]) @ W2[e] + b2[e],  e = component_idx[n]

Host: sort tokens by expert, split the 20 groups into 24 fragments, deal
into 8 cores x 3 "expert slots" with uniform per-slot capacity (SPMD:
every core runs the same static program; the host stages each slot's
expert weights/tokens). Slot order = [middle, largest, smallest]: slot0
starts as soon as its first per-dt slabs land; the fill (HBM-bound at
~320 GB/s) stays ahead of the PE for slots 1/2; the smallest slot plus
a 128-token tail chunk minimize the exit chain.

Device, per core:
 - All payload DMAs are issued in strict consumption order, alternating
   between the two HWDGE rings (scalar + sync sequencers). Weights for
   slots 1/2 stream as per-dt 196KB pieces just-in-time behind slot0's
   slabs. Every input tile has a unique pool tag so no dma_start ever
   blocks a sequencer on a far-future semaphore.
 - PE warmup: garbage fp16 matmuls sized so the HAM clock gate flips to
   2.4 GHz right as the first real slabs land.
 - Layer 1: 6x6 accumulating fp16 matmuls per chunk (<=512 tokens);
   slot0's first chunk runs dt-major so each round needs only one slab.
 - relu+bias: PSUM->SBUF fp16, split across Vector and Scalar engines.
 - Layer 2 (C=2): 6 accumulating [128->2] matmuls into a [2,T] PSUM
   bank, bias added on Vector. (A 4-column-group packed variant was
   measured at parity with this - the selector matmul eats the gain.)
"""

import math

import numpy as np

import concourse.bass as bass  # noqa: F401
import concourse.mybir as mybir
from concourse import bacc
from concourse.bass_utils import run_bass_kernel_spmd
from concourse.tile import TileContext

F32 = mybir.dt.float32
F16 = mybir.dt.float16
MM_DT = F16
MM_NP = np.float16

N_CORES = 8
N_SLOTS = 3
D = 768
H = 768
C = 2
DT = D // 128  # 6 d-tiles
HT = H // 128  # 6 h-tiles
MAX_CHUNK = 512  # one PSUM bank holds 512 fp32 -> matmul free dim cap
TAIL = 128  # final chunk size (shortens the exit chain)

N_WARMUP = 24  # fp16 [128,128] dummy matmuls until the first slabs land

ADD = mybir.AluOpType.add
MAX_OP = mybir.AluOpType.max
RELU = mybir.ActivationFunctionType.Relu


def _chunk_sizes(cap: int, tail_split: bool = False) -> list[int]:
    n = max(1, math.ceil(cap / MAX_CHUNK))
    base = (cap // n) & ~1
    sizes = [base] * n
    sizes[-1] = cap - base * (n - 1)
    if tail_split and sizes[-1] > 2 * TAIL:
        sizes[-1:] = [sizes[-1] - TAIL, TAIL]
    assert sum(sizes) == cap and all(s % 2 == 0 and 0 < s <= MAX_CHUNK for s in sizes)
    return sizes


def _plan_packing(counts: np.ndarray):
    """Return (caps, assign): per-slot capacities and
    assign[s][c] = (expert, start_within_group, length).
    Slot order: [middle..., largest, smallest]."""
    frags = [(int(e), 0, int(c)) for e, c in enumerate(counts) if c > 0]
    target = N_CORES * N_SLOTS
    assert len(frags) <= target, (
        f"{len(frags)} non-empty experts exceed {target} slots; raise N_SLOTS"
    )
    while len(frags) < target:
        frags.sort(key=lambda f: -f[2])
        e, st, ln = frags[0]
        if ln < 2:
            frags.append((e, st, 0))
            continue
        h1 = ln // 2
        frags[0] = (e, st, ln - h1)
        frags.append((e, st + (ln - h1), h1))
    frags.sort(key=lambda f: -f[2])
    groups = []
    for s in range(N_SLOTS):
        group = frags[s * N_CORES : (s + 1) * N_CORES]
        cap = max(2, max(f[2] for f in group))
        cap += cap % 2
        groups.append((cap, group))
    # middle slots first, then largest, smallest last
    groups.sort(key=lambda g: -g[0])
    largest, smallest = groups[0], groups[-1]
    middle = groups[1:-1]
    groups = middle + [largest, smallest]
    return [g[0] for g in groups], [g[1] for g in groups]


_PROGRAM_CACHE: dict = {}


def _build_program(caps: tuple):
    if caps in _PROGRAM_CACHE:
        return _PROGRAM_CACHE[caps]

    R = sum(caps)
    offs = [0]
    for c in caps[:-1]:
        offs.append(offs[-1] + c)
    chunk_plan = []  # (s, co, size, last_of_slot)
    for s in range(N_SLOTS):
        sizes = _chunk_sizes(caps[s], tail_split=(s == N_SLOTS - 1))
        co = 0
        for ci, size in enumerate(sizes):
            chunk_plan.append((s, co, size, ci == len(sizes) - 1))
            co += size
    n_chunks = len(chunk_plan)

    nc = bacc.Bacc(
        "TRN2", target_bir_lowering=False, debug=False, num_devices=N_CORES
    )
    xTp = nc.dram_tensor("xTp", [128, DT, R], MM_DT, kind="ExternalInput")
    w1 = nc.dram_tensor("w1", [N_SLOTS, 128, DT, H], MM_DT, kind="ExternalInput")
    b1 = nc.dram_tensor("b1", [128, N_SLOTS, HT], F32, kind="ExternalInput")
    w2 = nc.dram_tensor("w2", [128, N_SLOTS, HT, C], MM_DT, kind="ExternalInput")
    b2 = nc.dram_tensor("b2", [C, N_SLOTS], F32, kind="ExternalInput")
    y = nc.dram_tensor("y", [C, R], F32, kind="ExternalOutput")

    with TileContext(nc) as tc:
        with (
            tc.tile_pool(name="inpool", bufs=1) as inpool,
            tc.tile_pool(name="hpool", bufs=3) as hpool,
            tc.tile_pool(name="pspool", bufs=7, space="PSUM") as pspool,
            tc.tile_pool(name="pypool", bufs=1, space="PSUM") as pypool,
        ):
            # Strict consumption-order DMA. The scalar ring carries ONLY
            # slot0's six slabs: a DIRECT2D issue blocks the sequencer
            # when the HWDGE ring is full, and the scalar sequencer must
            # be free for relu work from ~15us on. Everything later rides
            # the sync ring (which never runs compute).
            def ring():
                return nc.sync

            # warmup constants first on the gpsimd queue (before its SWDGE
            # issue) so warmup matmuls can start by ~7.5us
            wu_w = inpool.tile([128, 128], MM_DT, name="wu_w", tag="wu_w")
            wu_x = inpool.tile([128, 128], MM_DT, name="wu_x", tag="wu_x")
            nc.gpsimd.memset(wu_w[:, :], 0.0)
            nc.gpsimd.memset(wu_x[:, :], 0.0)

            # primes: wake both rings' SDMA paths
            scr_a = inpool.tile([128, 64], MM_DT, name="scr_a", tag="scr_a")
            nc.scalar.dma_start(out=scr_a, in_=xTp[:, 0, 0:64])
            scr_b = inpool.tile([128, 64], MM_DT, name="scr_b", tag="scr_b")
            nc.sync.dma_start(out=scr_b, in_=w1[0, :, 0, 0:64])

            # slot0: per-dt w slabs + per-(dt,chunk) x pieces, opposite
            # rings per dt; chunk0's x piece rides with the w slab so the
            # first dt-major round needs only ~300KB in flight
            s0_sizes = _chunk_sizes(caps[0])
            s0_cuts = [0]
            for z in s0_sizes:
                s0_cuts.append(s0_cuts[-1] + z)
            s0w, s0x = [], []
            for dt in range(DT):
                wt = inpool.tile([128, H], MM_DT, name=f"w0d{dt}", tag=f"w0d{dt}")
                xt = inpool.tile(
                    [128, caps[0]], MM_DT, name=f"x0d{dt}", tag=f"x0d{dt}"
                )
                r_w = nc.scalar if dt % 2 == 0 else nc.sync
                r_x = nc.sync if dt % 2 == 0 else nc.scalar
                if dt == 0:
                    # the very first x piece rides SWDGE: the gpsimd
                    # sequencer enters main ~1us before the HWDGE ones,
                    # so this lands first and the PE starts earliest.
                    # w0d0 splits in half across both HWDGE rings so the
                    # first dt-round unblocks ~1us sooner.
                    r_x = nc.gpsimd
                    nc.scalar.dma_start(
                        out=wt[:, 0 : H // 2], in_=w1[0, :, 0, 0 : H // 2]
                    )
                    nc.sync.dma_start(
                        out=wt[:, H // 2 : H], in_=w1[0, :, 0, H // 2 : H]
                    )
                else:
                    r_w.dma_start(out=wt, in_=w1[0, :, dt, :])
                r_x.dma_start(
                    out=xt[:, 0 : s0_cuts[1]], in_=xTp[:, dt, 0 : s0_cuts[1]]
                )
                s0w.append(wt)
                s0x.append(xt)
            # later chunks' x pieces after all of chunk0's
            for ci in range(1, len(s0_sizes)):
                lo, hi = s0_cuts[ci], s0_cuts[ci + 1]
                for dt in range(DT):
                    r_x = nc.sync if dt % 2 == 0 else nc.scalar
                    r_x.dma_start(
                        out=s0x[dt][:, lo:hi], in_=xTp[:, dt, lo:hi]
                    )

            # (warmup constants are memset at the very top of the gpsimd
            # queue, before its SWDGE issue, so warmup matmuls start early)

            # small constants
            b1_sb = inpool.tile([128, N_SLOTS, HT], F32, name="b1_sb", tag="b1")
            ring().dma_start(out=b1_sb, in_=b1[:, :, :])
            w2_sb = inpool.tile([128, N_SLOTS, HT, C], MM_DT, name="w2_sb", tag="w2")
            ring().dma_start(out=w2_sb, in_=w2[:, :, :, :])
            b2_sb = inpool.tile([C, N_SLOTS], F32, name="b2_sb", tag="b2")
            ring().dma_start(out=b2_sb, in_=b2[:, :])

            # slots 1..: per-dt weight pieces (JIT) + whole-chunk x tiles,
            # interleaved in consumption order
            sw = {0: s0w}
            sx = {}
            for s in range(1, N_SLOTS):
                wt = inpool.tile(
                    [128, DT, H], MM_DT, name=f"wslot{s}", tag=f"wslot{s}"
                )
                x_first = [co for (ss, co, _, _) in chunk_plan if ss == s][0]
                for dt in range(DT):
                    ring().dma_start(out=wt[:, dt, :], in_=w1[s, :, dt, :])
                    if dt == 2:
                        # first x chunk of this slot lands mid-weight-stream
                        sz = [z for (ss, co, z, _) in chunk_plan if ss == s][0]
                        xt = inpool.tile(
                            [128, DT, sz], MM_DT, name=f"x{s}_0", tag=f"x{s}_0"
                        )
                        ring().dma_start(
                            out=xt, in_=xTp[:, :, offs[s] : offs[s] + sz]
                        )
                        sx[(s, 0)] = xt
                sw[s] = wt
                for ss, co, sz, _ in chunk_plan:
                    if ss != s or (ss, co) in sx:
                        continue
                    xt = inpool.tile(
                        [128, DT, sz], MM_DT, name=f"x{s}_{co}", tag=f"x{s}_{co}"
                    )
                    ring().dma_start(
                        out=xt, in_=xTp[:, :, offs[s] + co : offs[s] + co + sz]
                    )
                    sx[(s, co)] = xt

            # PE warmup: flips the HAM clock gate before real data lands
            wu_ps = pypool.tile([128, 128], F32, name="wu_ps", tag="psy")
            for _ in range(N_WARMUP):
                nc.tensor.matmul(wu_ps, wu_w, wu_x, start=True, stop=True)

            y_slot = [
                inpool.tile([C, caps[s]], F32, name=f"ysb{s}", tag=f"ysb{s}")
                for s in range(N_SLOTS)
            ]

            # ---- main loop ----
            for idx, (s, co, size, last) in enumerate(chunk_plan):
                is_final = idx == n_chunks - 1
                h_sb = hpool.tile([128, HT, size], MM_DT, name="h_sb", tag="h")
                ps_list = [
                    pspool.tile([128, size], F32, name=f"ps_h{ht}", tag="psh")
                    for ht in range(HT)
                ]
                if s == 0 and co == 0:
                    # dt-major: round dt needs only that dt's two slabs
                    for dt in range(DT):
                        for ht in range(HT):
                            nc.tensor.matmul(
                                ps_list[ht],
                                s0w[dt][:, ht * 128 : (ht + 1) * 128],
                                s0x[dt][:, co : co + size],
                                start=(dt == 0),
                                stop=(dt == DT - 1),
                            )
                elif s == 0:
                    for ht in range(HT):
                        for dt in range(DT):
                            nc.tensor.matmul(
                                ps_list[ht],
                                s0w[dt][:, ht * 128 : (ht + 1) * 128],
                                s0x[dt][:, co : co + size],
                                start=(dt == 0),
                                stop=(dt == DT - 1),
                            )
                else:
                    wt = sw[s]
                    xt = sx[(s, co)]
                    for ht in range(HT):
                        for dt in range(DT):
                            nc.tensor.matmul(
                                ps_list[ht],
                                wt[:, dt, ht * 128 : (ht + 1) * 128],
                                xt[:, dt, :],
                                start=(dt == 0),
                                stop=(dt == DT - 1),
                            )
                # relu+bias, split across both elementwise engines
                for ht in range(HT):
                    if ht % 2 == 0:
                        nc.vector.tensor_scalar(
                            h_sb[:, ht, :],
                            ps_list[ht],
                            b1_sb[:, s, ht : ht + 1],
                            0.0,
                            op0=ADD,
                            op1=MAX_OP,
                        )
                    else:
                        nc.scalar.activation(
                            h_sb[:, ht, :],
                            ps_list[ht],
                            RELU,
                            bias=b1_sb[:, s, ht : ht + 1],
                        )
                # layer 2 + bias
                ps_y = pypool.tile([C, size], F32, name="ps_y", tag="psy")
                for ht in range(HT):
                    nc.tensor.matmul(
                        ps_y,
                        w2_sb[:, s, ht, :],
                        h_sb[:, ht, :],
                        start=(ht == 0),
                        stop=(ht == HT - 1),
                    )
                nc.vector.tensor_scalar_add(
                    y_slot[s][:, co : co + size], ps_y, b2_sb[:, s : s + 1]
                )
                if s == N_SLOTS - 1:
                    nc.sync.dma_start(
                        out=y[:, offs[s] + co : offs[s] + co + size],
                        in_=y_slot[s][:, co : co + size],
                    )
                elif last:
                    nc.sync.dma_start(
                        out=y[:, offs[s] : offs[s] + caps[s]],
                        in_=y_slot[s][:, 0 : caps[s]],
                    )

    nc.compile()
    _PROGRAM_CACHE[caps] = nc
    return nc


def kernel(embeddings, component_idx, W1, b1, W2, b2):
    embeddings = np.ascontiguousarray(np.asarray(embeddings, dtype=np.float32))
    ci = np.asarray(component_idx).astype(np.int64, copy=False)
    W1 = np.asarray(W1, dtype=np.float32)
    b1 = np.asarray(b1, dtype=np.float32)
    W2 = np.asarray(W2, dtype=np.float32)
    b2 = np.asarray(b2, dtype=np.float32)

    N = embeddings.shape[0]
    E = W1.shape[0]

    counts = np.bincount(ci, minlength=E)
    order = np.argsort(ci, kind="stable")
    group_start = np.zeros(E, dtype=np.int64)
    group_start[1:] = np.cumsum(counts)[:-1]
    x_sorted = embeddings[order]  # [N, D] grouped by expert

    caps, assign = _plan_packing(counts)
    R = sum(caps)
    offs = np.cumsum([0] + caps[:-1]).tolist() if len(caps) > 1 else [0]

    nc = _build_program(tuple(caps))

    # host-side packing
    w1_packed = np.ascontiguousarray(
        W1.reshape(E, DT, 128, H).transpose(0, 2, 1, 3)
    ).astype(MM_NP)  # [e, p, dt, h]
    b1_packed = np.ascontiguousarray(
        b1.reshape(E, HT, 128).transpose(0, 2, 1)
    )  # [e, 128, ht]
    w2_packed = np.ascontiguousarray(
        W2.reshape(E, HT, 128, C).transpose(0, 2, 1, 3)
    ).astype(MM_NP)  # [e, p, ht, c]

    in_maps = []
    for c in range(N_CORES):
        Xc = np.zeros((R, D), dtype=MM_NP)
        w1_in = np.empty((N_SLOTS, 128, DT, H), dtype=MM_NP)
        b1_in = np.empty((128, N_SLOTS, HT), dtype=np.float32)
        w2_in = np.empty((128, N_SLOTS, HT, C), dtype=MM_NP)
        b2_in = np.empty((C, N_SLOTS), dtype=np.float32)
        for s in range(N_SLOTS):
            e, st, ln = assign[s][c]
            beg = group_start[e] + st
            Xc[offs[s] : offs[s] + ln] = x_sorted[beg : beg + ln]
            w1_in[s] = w1_packed[e]
            b1_in[:, s, :] = b1_packed[e]
            w2_in[:, s, :, :] = w2_packed[e]
            b2_in[:, s] = b2[e]
        xTp_in = np.ascontiguousarray(Xc.T.reshape(DT, 128, R).transpose(1, 0, 2))
        im = {"xTp": xTp_in, "w1": w1_in, "b1": b1_in, "w2": w2_in, "b2": b2_in}
        in_maps.append(im)

    global _LAST_IN_MAPS
    _LAST_IN_MAPS = in_maps
    res = run_bass_kernel_spmd(nc, in_maps, list(range(N_CORES)))

    out = np.empty((N, C), dtype=np.float32)
    for c in range(N_CORES):
        yc = res.results[c]["y"]  # [C, R]
        for s in range(N_SLOTS):
            e, st, ln = assign[s][c]
            beg = group_start[e] + st
            tokens = order[beg : beg + ln]
            out[tokens] = yc[:, offs[s] : offs[s] + ln].T
    return out


# revision 25
# speedup vs baseline: 1.0029x; 1.0029x over previous
"""MoE routed-expert kernel for Trainium2 (8 NeuronCores, SPMD).

Problem: N=16384 tokens, D=768, H=768, C=2, E=20 experts.
  y[n] = relu(x[n] @ W1[e] + b1[e]) @ W2[e] + b2[e],  e = component_idx[n]

Host: sort tokens by expert, split the 20 groups into 24 fragments, deal
into 8 cores x 3 "expert slots" with uniform per-slot capacity (SPMD:
every core runs the same static program; the host stages each slot's
expert weights/tokens). Slot order = [middle, largest, smallest]: slot0
starts as soon as its first per-dt slabs land; the fill (HBM-bound at
~320 GB/s) stays ahead of the PE for slots 1/2; the smallest slot plus
a 128-token tail chunk minimize the exit chain.

Device, per core:
 - All payload DMAs are issued in strict consumption order, alternating
   between the two HWDGE rings (scalar + sync sequencers). Weights for
   slots 1/2 stream as per-dt 196KB pieces just-in-time behind slot0's
   slabs. Every input tile has a unique pool tag so no dma_start ever
   blocks a sequencer on a far-future semaphore.
 - PE warmup: garbage fp16 matmuls sized so the HAM clock gate flips to
   2.4 GHz right as the first real slabs land.
 - Layer 1: 6x6 accumulating fp16 matmuls per chunk (<=512 tokens);
   slot0's first chunk runs dt-major so each round needs only one slab.
 - relu+bias: PSUM->SBUF fp16, split across Vector and Scalar engines.
 - Layer 2 (C=2): 6 accumulating [128->2] matmuls into a [2,T] PSUM
   bank, bias added on Vector. (A 4-column-group packed variant was
   measured at parity with this - the selector matmul eats the gain.)
"""

import math

import numpy as np

import concourse.bass as bass  # noqa: F401
import concourse.mybir as mybir
from concourse import bacc
from concourse.bass_utils import run_bass_kernel_spmd
from concourse.tile import TileContext

F32 = mybir.dt.float32
F16 = mybir.dt.float16
MM_DT = F16
MM_NP = np.float16

N_CORES = 8
N_SLOTS = 3
D = 768
H = 768
C = 2
DT = D // 128  # 6 d-tiles
HT = H // 128  # 6 h-tiles
MAX_CHUNK = 512  # one PSUM bank holds 512 fp32 -> matmul free dim cap
TAIL = 128  # final chunk size (shortens the exit chain)

N_WARMUP = 24  # fp16 [128,128] dummy matmuls until the first slabs land

ADD = mybir.AluOpType.add
MAX_OP = mybir.AluOpType.max
RELU = mybir.ActivationFunctionType.Relu


def _chunk_sizes(cap: int, tail_split: bool = False) -> list[int]:
    n = max(1, math.ceil(cap / MAX_CHUNK))
    base = (cap // n) & ~1
    sizes = [base] * n
    sizes[-1] = cap - base * (n - 1)
    if tail_split and sizes[-1] > 2 * TAIL:
        sizes[-1:] = [sizes[-1] - TAIL, TAIL]
    assert sum(sizes) == cap and all(s % 2 == 0 and 0 < s <= MAX_CHUNK for s in sizes)
    return sizes


def _plan_packing(counts: np.ndarray):
    """Return (caps, assign): per-slot capacities and
    assign[s][c] = (expert, start_within_group, length).
    Slot order: [middle..., largest, smallest]."""
    frags = [(int(e), 0, int(c)) for e, c in enumerate(counts) if c > 0]
    target = N_CORES * N_SLOTS
    assert len(frags) <= target, (
        f"{len(frags)} non-empty experts exceed {target} slots; raise N_SLOTS"
    )
    while len(frags) < target:
        frags.sort(key=lambda f: -f[2])
        e, st, ln = frags[0]
        if ln < 2:
            frags.append((e, st, 0))
            continue
        h1 = ln // 2
        frags[0] = (e, st, ln - h1)
        frags.append((e, st + (ln - h1), h1))
    frags.sort(key=lambda f: -f[2])
    groups = []
    for s in range(N_SLOTS):
        group = frags[s * N_CORES : (s + 1) * N_CORES]
        cap = max(2, max(f[2] for f in group))
        cap += cap % 2
        groups.append((cap, group))
    # middle slots first, then largest, smallest last
    groups.sort(key=lambda g: -g[0])
    largest, smallest = groups[0], groups[-1]
    middle = groups[1:-1]
    groups = middle + [largest, smallest]
    return [g[0] for g in groups], [g[1] for g in groups]


_PROGRAM_CACHE: dict = {}


def _build_program(caps: tuple):
    if caps in _PROGRAM_CACHE:
        return _PROGRAM_CACHE[caps]

    R = sum(caps)
    offs = [0]
    for c in caps[:-1]:
        offs.append(offs[-1] + c)
    chunk_plan = []  # (s, co, size, last_of_slot)
    for s in range(N_SLOTS):
        sizes = _chunk_sizes(caps[s], tail_split=(s == N_SLOTS - 1))
        co = 0
        for ci, size in enumerate(sizes):
            chunk_plan.append((s, co, size, ci == len(sizes) - 1))
            co += size
    n_chunks = len(chunk_plan)

    nc = bacc.Bacc(
        "TRN2", target_bir_lowering=False, debug=False, num_devices=N_CORES
    )
    xTp = nc.dram_tensor("xTp", [128, DT, R], MM_DT, kind="ExternalInput")
    w1 = nc.dram_tensor("w1", [N_SLOTS, 128, DT, H], MM_DT, kind="ExternalInput")
    b1 = nc.dram_tensor("b1", [128, N_SLOTS, HT], F32, kind="ExternalInput")
    w2 = nc.dram_tensor("w2", [128, N_SLOTS, HT, C], MM_DT, kind="ExternalInput")
    b2 = nc.dram_tensor("b2", [C, N_SLOTS], F32, kind="ExternalInput")
    y = nc.dram_tensor("y", [C, R], F32, kind="ExternalOutput")

    with TileContext(nc) as tc:
        with (
            tc.tile_pool(name="inpool", bufs=1) as inpool,
            tc.tile_pool(name="hpool", bufs=3) as hpool,
            tc.tile_pool(name="pspool", bufs=7, space="PSUM") as pspool,
            tc.tile_pool(name="pypool", bufs=1, space="PSUM") as pypool,
        ):
            # Strict consumption-order DMA. The scalar ring carries ONLY
            # slot0's six slabs: a DIRECT2D issue blocks the sequencer
            # when the HWDGE ring is full, and the scalar sequencer must
            # be free for relu work from ~15us on. Everything later rides
            # the sync ring (which never runs compute).
            def ring():
                return nc.sync

            # warmup constants first on the gpsimd queue (before its SWDGE
            # issue) so warmup matmuls can start by ~7.5us
            wu_w = inpool.tile([128, 128], MM_DT, name="wu_w", tag="wu_w")
            wu_x = inpool.tile([128, 128], MM_DT, name="wu_x", tag="wu_x")
            nc.gpsimd.memset(wu_w[:, :], 0.0)
            nc.gpsimd.memset(wu_x[:, :], 0.0)

            # slot0: per-dt w slabs + per-(dt,chunk) x pieces, opposite
            # rings per dt; chunk0's x piece rides with the w slab so the
            # first dt-major round needs only ~300KB in flight
            s0_sizes = _chunk_sizes(caps[0])
            s0_cuts = [0]
            for z in s0_sizes:
                s0_cuts.append(s0_cuts[-1] + z)
            s0w, s0x = [], []
            for dt in range(DT):
                wt = inpool.tile([128, H], MM_DT, name=f"w0d{dt}", tag=f"w0d{dt}")
                xt = inpool.tile(
                    [128, caps[0]], MM_DT, name=f"x0d{dt}", tag=f"x0d{dt}"
                )
                r_w = nc.scalar if dt % 2 == 0 else nc.sync
                r_x = nc.sync if dt % 2 == 0 else nc.scalar
                if dt == 0:
                    # the very first x piece rides SWDGE: the gpsimd
                    # sequencer enters main ~1us before the HWDGE ones,
                    # so this lands first and the PE starts earliest.
                    # w0d0 splits in half across both HWDGE rings so the
                    # first dt-round unblocks ~1us sooner.
                    r_x = nc.gpsimd
                    nc.scalar.dma_start(
                        out=wt[:, 0 : H // 2], in_=w1[0, :, 0, 0 : H // 2]
                    )
                    nc.sync.dma_start(
                        out=wt[:, H // 2 : H], in_=w1[0, :, 0, H // 2 : H]
                    )
                else:
                    r_w.dma_start(out=wt, in_=w1[0, :, dt, :])
                r_x.dma_start(
                    out=xt[:, 0 : s0_cuts[1]], in_=xTp[:, dt, 0 : s0_cuts[1]]
                )
                s0w.append(wt)
                s0x.append(xt)
            # later chunks' x pieces after all of chunk0's
            for ci in range(1, len(s0_sizes)):
                lo, hi = s0_cuts[ci], s0_cuts[ci + 1]
                for dt in range(DT):
                    r_x = nc.sync if dt % 2 == 0 else nc.scalar
                    r_x.dma_start(
                        out=s0x[dt][:, lo:hi], in_=xTp[:, dt, lo:hi]
                    )

            # (warmup constants are memset at the very top of the gpsimd
            # queue, before its SWDGE issue, so warmup matmuls start early)

            # small constants
            b1_sb = inpool.tile([128, N_SLOTS, HT], F32, name="b1_sb", tag="b1")
            ring().dma_start(out=b1_sb, in_=b1[:, :, :])
            w2_sb = inpool.tile([128, N_SLOTS, HT, C], MM_DT, name="w2_sb", tag="w2")
            ring().dma_start(out=w2_sb, in_=w2[:, :, :, :])
            b2_sb = inpool.tile([C, N_SLOTS], F32, name="b2_sb", tag="b2")
            ring().dma_start(out=b2_sb, in_=b2[:, :])

            # slots 1..: per-dt weight pieces (JIT) + whole-chunk x tiles,
            # interleaved in consumption order
            sw = {0: s0w}
            sx = {}
            for s in range(1, N_SLOTS):
                wt = inpool.tile(
                    [128, DT, H], MM_DT, name=f"wslot{s}", tag=f"wslot{s}"
                )
                x_first = [co for (ss, co, _, _) in chunk_plan if ss == s][0]
                for dt in range(DT):
                    ring().dma_start(out=wt[:, dt, :], in_=w1[s, :, dt, :])
                    if dt == 2:
                        # first x chunk of this slot lands mid-weight-stream
                        sz = [z for (ss, co, z, _) in chunk_plan if ss == s][0]
                        xt = inpool.tile(
                            [128, DT, sz], MM_DT, name=f"x{s}_0", tag=f"x{s}_0"
                        )
                        ring().dma_start(
                            out=xt, in_=xTp[:, :, offs[s] : offs[s] + sz]
                        )
                        sx[(s, 0)] = xt
                sw[s] = wt
                for ss, co, sz, _ in chunk_plan:
                    if ss != s or (ss, co) in sx:
                        continue
                    xt = inpool.tile(
                        [128, DT, sz], MM_DT, name=f"x{s}_{co}", tag=f"x{s}_{co}"
                    )
                    ring().dma_start(
                        out=xt, in_=xTp[:, :, offs[s] + co : offs[s] + co + sz]
                    )
                    sx[(s, co)] = xt

            # PE warmup: flips the HAM clock gate before real data lands
            wu_ps = pypool.tile([128, 128], F32, name="wu_ps", tag="psy")
            for _ in range(N_WARMUP):
                nc.tensor.matmul(wu_ps, wu_w, wu_x, start=True, stop=True)

            y_slot = [
                inpool.tile([C, caps[s]], F32, name=f"ysb{s}", tag=f"ysb{s}")
                for s in range(N_SLOTS)
            ]

            # ---- main loop ----
            for idx, (s, co, size, last) in enumerate(chunk_plan):
                is_final = idx == n_chunks - 1
                h_sb = hpool.tile([128, HT, size], MM_DT, name="h_sb", tag="h")
                ps_list = [
                    pspool.tile([128, size], F32, name=f"ps_h{ht}", tag="psh")
                    for ht in range(HT)
                ]
                if s == 0 and co == 0:
                    # dt-major: round dt needs only that dt's two slabs
                    for dt in range(DT):
                        for ht in range(HT):
                            nc.tensor.matmul(
                                ps_list[ht],
                                s0w[dt][:, ht * 128 : (ht + 1) * 128],
                                s0x[dt][:, co : co + size],
                                start=(dt == 0),
                                stop=(dt == DT - 1),
                            )
                elif s == 0:
                    for ht in range(HT):
                        for dt in range(DT):
                            nc.tensor.matmul(
                                ps_list[ht],
                                s0w[dt][:, ht * 128 : (ht + 1) * 128],
                                s0x[dt][:, co : co + size],
                                start=(dt == 0),
                                stop=(dt == DT - 1),
                            )
                else:
                    wt = sw[s]
                    xt = sx[(s, co)]
                    for ht in range(HT):
                        for dt in range(DT):
                            nc.tensor.matmul(
                                ps_list[ht],
                                wt[:, dt, ht * 128 : (ht + 1) * 128],
                                xt[:, dt, :],
                                start=(dt == 0),
                                stop=(dt == DT - 1),
                            )
                # relu+bias, split across both elementwise engines
                for ht in range(HT):
                    if ht % 2 == 0:
                        nc.vector.tensor_scalar(
                            h_sb[:, ht, :],
                            ps_list[ht],
                            b1_sb[:, s, ht : ht + 1],
                            0.0,
                            op0=ADD,
                            op1=MAX_OP,
                        )
                    else:
                        nc.scalar.activation(
                            h_sb[:, ht, :],
                            ps_list[ht],
                            RELU,
                            bias=b1_sb[:, s, ht : ht + 1],
                        )
                # layer 2: alternate two PE column groups so consecutive
                # matmuls stream concurrently (~3T instead of 6T) and
                # their weight loads hide; fold the two [2,T] partials +
                # bias with two DVE ops. Final chunk stays single-group
                # (shortest exit chain).
                ps_y = pypool.tile([128, size], F32, name="ps_y", tag="psy")
                n_grp = 1 if is_final else 2
                for ht in range(HT):
                    g = ht % n_grp
                    nc.tensor.matmul(
                        ps_y[32 * g : 32 * g + C, :],
                        w2_sb[:, s, ht, :],
                        h_sb[:, ht, :],
                        start=(ht < n_grp),
                        stop=(ht >= HT - n_grp),
                        tile_position=(0, 32 * g),
                    )
                nc.vector.tensor_scalar_add(
                    y_slot[s][:, co : co + size],
                    ps_y[0:C, :],
                    b2_sb[:, s : s + 1],
                )
                if n_grp == 2:
                    nc.vector.tensor_tensor(
                        y_slot[s][:, co : co + size],
                        y_slot[s][:, co : co + size],
                        ps_y[32 : 32 + C, :],
                        op=ADD,
                    )
                if s == N_SLOTS - 1:
                    nc.sync.dma_start(
                        out=y[:, offs[s] + co : offs[s] + co + size],
                        in_=y_slot[s][:, co : co + size],
                    )
                elif last:
                    nc.sync.dma_start(
                        out=y[:, offs[s] : offs[s] + caps[s]],
                        in_=y_slot[s][:, 0 : caps[s]],
                    )

    nc.compile()
    _PROGRAM_CACHE[caps] = nc
    return nc


def kernel(embeddings, component_idx, W1, b1, W2, b2):
    embeddings = np.ascontiguousarray(np.asarray(embeddings, dtype=np.float32))
    ci = np.asarray(component_idx).astype(np.int64, copy=False)
    W1 = np.asarray(W1, dtype=np.float32)
    b1 = np.asarray(b1, dtype=np.float32)
    W2 = np.asarray(W2, dtype=np.float32)
    b2 = np.asarray(b2, dtype=np.float32)

    N = embeddings.shape[0]
    E = W1.shape[0]

    counts = np.bincount(ci, minlength=E)
    order = np.argsort(ci, kind="stable")
    group_start = np.zeros(E, dtype=np.int64)
    group_start[1:] = np.cumsum(counts)[:-1]
    x_sorted = embeddings[order]  # [N, D] grouped by expert

    caps, assign = _plan_packing(counts)
    R = sum(caps)
    offs = np.cumsum([0] + caps[:-1]).tolist() if len(caps) > 1 else [0]

    nc = _build_program(tuple(caps))

    # host-side packing
    w1_packed = np.ascontiguousarray(
        W1.reshape(E, DT, 128, H).transpose(0, 2, 1, 3)
    ).astype(MM_NP)  # [e, p, dt, h]
    b1_packed = np.ascontiguousarray(
        b1.reshape(E, HT, 128).transpose(0, 2, 1)
    )  # [e, 128, ht]
    w2_packed = np.ascontiguousarray(
        W2.reshape(E, HT, 128, C).transpose(0, 2, 1, 3)
    ).astype(MM_NP)  # [e, p, ht, c]

    in_maps = []
    for c in range(N_CORES):
        Xc = np.zeros((R, D), dtype=MM_NP)
        w1_in = np.empty((N_SLOTS, 128, DT, H), dtype=MM_NP)
        b1_in = np.empty((128, N_SLOTS, HT), dtype=np.float32)
        w2_in = np.empty((128, N_SLOTS, HT, C), dtype=MM_NP)
        b2_in = np.empty((C, N_SLOTS), dtype=np.float32)
        for s in range(N_SLOTS):
            e, st, ln = assign[s][c]
            beg = group_start[e] + st
            Xc[offs[s] : offs[s] + ln] = x_sorted[beg : beg + ln]
            w1_in[s] = w1_packed[e]
            b1_in[:, s, :] = b1_packed[e]
            w2_in[:, s, :, :] = w2_packed[e]
            b2_in[:, s] = b2[e]
        xTp_in = np.ascontiguousarray(Xc.T.reshape(DT, 128, R).transpose(1, 0, 2))
        im = {"xTp": xTp_in, "w1": w1_in, "b1": b1_in, "w2": w2_in, "b2": b2_in}
        in_maps.append(im)

    global _LAST_IN_MAPS
    _LAST_IN_MAPS = in_maps
    res = run_bass_kernel_spmd(nc, in_maps, list(range(N_CORES)))

    out = np.empty((N, C), dtype=np.float32)
    for c in range(N_CORES):
        yc = res.results[c]["y"]  # [C, R]
        for s in range(N_SLOTS):
            e, st, ln = assign[s][c]
            beg = group_start[e] + st
            tokens = order[beg : beg + ln]
            out[tokens] = yc[:, offs[s] : offs[s] + ln].T
    return out
